# revision 31
# baseline (speedup 1.0000x reference)
"""ECC / GNN message-passing kernel for 8 Trainium2 NeuronCores (Bass/Tile).

Reference computation (per edge type t of 7):
    h   = x @ W[t] + b[t]                      # [B, N, F] dense projection
    msg = adj[t] * h[:, end_nodes[t], :]       # gather + edge scale
    out[:, start_nodes[t], :] += msg           # scatter-add

This kernel commutes the linear projection past the gather/scatter:
    out = sum_t (S_t @ x) @ W[t] + deg_t (x) b[t]
where S_t is the weighted adjacency (S_t[s,e] = sum adj over edges s<-e) and
deg_t = S_t @ 1. The deg (x) b bias term is precomputed on the host (O(E)
scalar work); everything O(E*F) runs on device.

Sharding: destination (start-node) range partitioning, 12500 dests/core.
Per core:
  - x is stored batch-interleaved ([N, 2F], 1KB rows) so a single dma_gather
    descriptor fetches both batches' features for an end node (the Q7
    descriptor generator is the serial bottleneck; this halves it).
  - dma_gather uses int16 indices, so x is split into 5 windows of 20000
    rows; edges are grouped (window, dest-block, type) and padded to
    128-edge chunks (pad edges get weight 0).
  - Scatter-add runs on the TensorEngine: per chunk, a [128e x 128d] one-hot
    (built on DVE: (iota == start_local) * adj) right-multiplies the gathered
    rows, accumulating y_T[f, d] per (dest-block, type) in PSUM.
  - Per (block, type): y_T is copied to SBUF (ScalarE) and folded with W[t]
    into PSUM out[d, f], then bias is added and the block is stored.

All arithmetic is fp32 (PE fp32 two-pass); relative error vs the fp32
reference is ~3e-7.
"""
import os
import numpy as np

import concourse.bass as bass
import concourse.mybir as mybir
import concourse.tile as tile
from concourse import bacc
from concourse._compat import get_trn_type
from concourse.bass_utils import run_bass_kernel_spmd

B, N, E, F, T = 2, 100000, 400000, 128, 7
CORES = 8
NPC = N // CORES            # 12500 dests per core
NBLK = (NPC + 127) // 128   # 98 dest blocks per core
NPAD = NBLK * 128
NW = 5                      # gather windows (int16 idx limit is 32768 rows)
WS = N // NW                # 20000
CALLCH = int(os.environ.get("K_CALLCH", "20"))  # chunks per dma_gather call


def _pack_inputs(x, W, b, adj_values, end_nodes, start_nodes):
    """Group edges by (core, window, dest-block, type); build the shared
    chunk structure (max over cores => one SPMD program), per-core gather
    indices, one-hot scalar columns, and the bias term."""
    starts = start_nodes.astype(np.int64).ravel()
    ends = end_nodes.astype(np.int64).ravel()
    adj = adj_values.astype(np.float32).ravel()
    types = np.repeat(np.arange(T, dtype=np.int64), E)

    core = starts // NPC
    sl_all = starts % NPC
    blk = sl_all // 128
    sl_blk = sl_all % 128
    win = ends // WS
    idx16 = (ends % WS).astype(np.int16)

    ng_pc = NW * NBLK * T
    gid_local = (win * NBLK + blk) * T + types
    gid = core * ng_pc + gid_local
    counts = np.bincount(gid, minlength=CORES * ng_pc).reshape(CORES, NW, NBLK, T)
    c_cnt = np.ceil(counts.max(axis=0) / 128).astype(np.int64)  # [NW, NBLK, T]

    chunk_off = np.zeros(ng_pc + 1, np.int64)
    np.cumsum(c_cnt.reshape(-1), out=chunk_off[1:])
    total_chunks = int(chunk_off[-1])
    total_slots = total_chunks * 128

    order = np.argsort(gid, kind="stable")
    sorted_key = gid[order]
    newrun = np.empty(len(sorted_key), bool)
    newrun[0] = True
    newrun[1:] = sorted_key[1:] != sorted_key[:-1]
    run_ids = np.cumsum(newrun) - 1
    run_first = np.flatnonzero(newrun)
    rank = np.arange(len(sorted_key)) - run_first[run_ids]
    slot_sorted = chunk_off[gid_local[order]] * 128 + rank

    # Pad slots get weight 0; spread their gather rows across the window so
    # the padding descriptors don't all hammer the same HBM line.
    pad_rows = (np.arange(total_slots, dtype=np.int64) % WS).astype(np.int16)
    idx_flat = np.broadcast_to(pad_rows, (CORES, total_slots)).copy()
    a_flat = np.zeros((CORES, total_slots), np.float16)
    sl_flat = np.zeros((CORES, total_slots), np.float16)
    core_sorted = core[order]
    idx_flat[core_sorted, slot_sorted] = idx16[order]
    a_flat[core_sorted, slot_sorted] = adj[order].astype(np.float16)
    sl_flat[core_sorted, slot_sorted] = sl_blk[order].astype(np.float16)

    # dma_gather index layout: idx j of a call -> [j % 16, col0 + j // 16],
    # replicated across the 8 groups of 16 partitions.
    idx_wrap = idx_flat.reshape(CORES, total_slots // 16, 16).transpose(0, 2, 1)
    idx_arr = np.tile(idx_wrap, (1, 8, 1)).copy()
    sl_cols = sl_flat.reshape(CORES, total_chunks, 128).transpose(0, 2, 1).copy()
    a_cols = a_flat.reshape(CORES, total_chunks, 128).transpose(0, 2, 1).copy()

    bias_full = np.zeros((N, F), np.float32)
    for t in range(T):
        deg = np.bincount(start_nodes[t].astype(np.int64),
                          weights=adj_values[t].astype(np.float64),
                          minlength=N).astype(np.float32)
        bias_full += deg[:, None] * b[t][None, :]
    # Transposed per-core bias [F, NPAD] (output is produced feature-major).
    biasT_pc = np.zeros((CORES, F, NPAD), np.float32)
    biasT_pc[:, :, :NPC] = bias_full.reshape(CORES, NPC, F).transpose(0, 2, 1)

    iota_rows = np.tile(np.arange(128, dtype=np.float16), (128, 1))

    x_il = np.ascontiguousarray(
        np.concatenate([x[0], x[1]], axis=1).astype(np.float16))  # [N, 2F]

    struct = dict(c_cnt=c_cnt, chunk_off=chunk_off, total_chunks=total_chunks)
    per_core = []
    for c in range(CORES):
        per_core.append({
            "x": x_il,
            "Wt": np.ascontiguousarray(W.astype(np.float16)),
            "idx": idx_arr[c],
            "sl": sl_cols[c],
            "av": a_cols[c],
            "bias": biasT_pc[c],
            "iota": iota_rows,
        })
    return struct, per_core


def build_kernel(c_cnt, chunk_off, total_chunks):
    DT = mybir.dt.float32
    H = mybir.dt.float16
    total_slots = total_chunks * 128
    nc = bacc.Bacc(get_trn_type() or "TRN2", target_bir_lowering=False,
                   num_swdge_queues=4)

    x_d = nc.dram_tensor("x", [N, B * F], H, kind="ExternalInput")
    W_d = nc.dram_tensor("Wt", [T, F, F], H, kind="ExternalInput")
    idx_d = nc.dram_tensor("idx", [128, total_slots // 16], mybir.dt.int16,
                           kind="ExternalInput")
    sl_d = nc.dram_tensor("sl", [128, total_chunks], H, kind="ExternalInput")
    a_d = nc.dram_tensor("av", [128, total_chunks], H, kind="ExternalInput")
    bias_d = nc.dram_tensor("bias", [F, NPAD], DT, kind="ExternalInput")
    iota_d = nc.dram_tensor("iota", [128, 128], H, kind="ExternalInput")
    # Output is feature-major [B, F, NPAD]; the host transposes back.
    out_d = nc.dram_tensor("out", [B, F, NPAD], DT, kind="ExternalOutput")

    def coff(ww, bb, tt):
        return int(chunk_off[(ww * NBLK + bb) * T + tt])

    # Per-window gather-call ranges aligned to dest-block boundaries so each
    # block reads exactly one call per window (bounds the live g-tile set).
    calls, call_of_blk = [], []
    for ww in range(NW):
        cs, cmap = [], []
        cur0 = coff(ww, 0, 0)
        for bbx in range(NBLK):
            b1 = coff(ww, bbx + 1, 0) if bbx + 1 < NBLK else (
                coff(ww + 1, 0, 0) if ww + 1 < NW else total_chunks)
            b0 = coff(ww, bbx, 0)
            if b1 - cur0 > CALLCH and b0 > cur0:
                cs.append((cur0, b0))
                cur0 = b0
            cmap.append(len(cs))
        last1 = coff(ww + 1, 0, 0) if ww + 1 < NW else total_chunks
        if last1 > cur0:
            cs.append((cur0, last1))
        cmap = [min(ci, len(cs) - 1) for ci in cmap]
        calls.append(cs)
        call_of_blk.append(cmap)

    G_BUFS = int(os.environ.get("K_GBUFS", "10"))
    MODE = os.environ.get("K_MODE", "full")  # full | gather_only | compute_only
    SB = 4                       # dest blocks folded per batched fold matmul
    NSB = (NBLK + SB - 1) // SB  # superblocks per core
    qrr = [0]

    with tile.TileContext(nc) as tc:
        with (
            tc.tile_pool(name="const", bufs=1) as cpool,
            tc.tile_pool(name="ip", bufs=12) as idxpool,
            tc.tile_pool(name="gp", bufs=G_BUFS) as gpool,
            tc.tile_pool(name="oh", bufs=int(os.environ.get("K_OHBUFS", "10"))) as ohpool,
            tc.tile_pool(name="yt", bufs=4) as ytpool,
            tc.tile_pool(name="ys", bufs=4) as yspool,
            tc.tile_pool(name="ev", bufs=6) as evpool,
            tc.tile_pool(name="py", bufs=int(os.environ.get("K_PYBUFS", "4")), space="PSUM") as pypool,
            tc.tile_pool(name="po", bufs=int(os.environ.get("K_POBUFS", "4")), space="PSUM") as popool,
        ):
            sl_t = cpool.tile([128, total_chunks], H)
            a_t = cpool.tile([128, total_chunks], H)
            iota_t = cpool.tile([128, 128], H)
            nc.sync.dma_start(out=sl_t[:], in_=sl_d[:])
            nc.sync.dma_start(out=a_t[:], in_=a_d[:])
            nc.sync.dma_start(out=iota_t[:], in_=iota_d[:])
            W_tiles = []
            for t in range(T):
                wt = cpool.tile([F, F], H, tag=f"W{t}")
                nc.sync.dma_start(out=wt[:], in_=W_d[t])
                W_tiles.append(wt)

            gt = {}

            fake_g = None
            if MODE == "pe_only":
                fake_g = cpool.tile([128, B * F], H, tag="fakeg")
                nc.sync.dma_start(out=fake_g[:], in_=x_d[0:128, :])

            def emit_call(ww, ci):
                c0, c1 = calls[ww][ci]
                nch = c1 - c0
                L = nch * 128
                if MODE == "pe_only":
                    gt[(ww, ci)] = True
                    return
                g = gpool.tile([128, nch, B * F], H, tag="g",
                               name=f"g{ww}_{ci}")
                if MODE == "compute_only":
                    r0 = (c0 * 128) % (N - L)
                    for ch in range(nch):
                        nc.sync.dma_start(
                            out=g[:, ch, :],
                            in_=x_d[r0 + ch * 128:r0 + (ch + 1) * 128, :])
                else:
                    idxp = idxpool.tile([128, nch * 8], mybir.dt.int16,
                                        tag="idxc", name=f"idxc{ww}_{ci}")
                    nc.sync.dma_start(out=idxp[:], in_=idx_d[:, c0 * 8:c1 * 8])
                    nc.gpsimd.dma_gather(
                        g[:],
                        x_d[ww * WS:(ww + 1) * WS, :],
                        idxp[:],
                        L, L, B * F,
                        queue_num=qrr[0] % 4,
                        single_packet=False,
                    )
                qrr[0] += 1
                gt[(ww, ci)] = g

            def chunk_rhs(ww, bb_, cglob):
                if MODE == "pe_only":
                    return fake_g[:]
                ci = call_of_blk[ww][bb_]
                c0 = calls[ww][ci][0]
                return gt[(ww, ci)][:, cglob - c0, :]

            # One-hot tiles are built per (block, window): the chunk columns
            # of one block within one window are contiguous in chunk space,
            # so 2 broadcast tensor_tensor ops build all of them at once.
            def build_oh(bb, ww):
                c0 = coff(ww, bb, 0)
                c1 = coff(ww, bb + 1, 0) if bb + 1 < NBLK else (
                    coff(ww + 1, 0, 0) if ww + 1 < NW else total_chunks)
                nch = c1 - c0
                oh = ohpool.tile([128, nch, 128], H, tag="oh",
                                 name=f"oh{bb}_{ww}")
                shp = (128, nch, 128)
                nc.vector.tensor_tensor(
                    out=oh[:],
                    in0=iota_t[:].unsqueeze(1).broadcast_to(shp),
                    in1=sl_t[:, c0:c1].unsqueeze(2).broadcast_to(shp),
                    op=mybir.AluOpType.is_equal)
                nc.vector.tensor_tensor(
                    out=oh[:], in0=oh[:],
                    in1=a_t[:, c0:c1].unsqueeze(2).broadcast_to(shp),
                    op=mybir.AluOpType.mult)
                return oh, c0

            for sb in range(NSB):
                blks = range(sb * SB, min((sb + 1) * SB, NBLK))
                sbw = len(blks)
                # yTcat[f, t, j, d]: per-type transposed partial sums for the
                # sbw blocks of this superblock, consumed by the batched fold.
                ytcat = {bt: ytpool.tile([F, T, sbw, 128], H, tag="ytc",
                                         name=f"ytc{sb}_{bt}")
                         for bt in range(B)}
                for bb in blks:
                    for ww in range(NW):
                        ci = call_of_blk[ww][bb]
                        if (ww, ci) not in gt:
                            emit_call(ww, ci)
                    ohs = {}
                    if MODE not in ("gather_only", "pe_only"):
                        for ww in range(NW):
                            ohs[ww] = build_oh(bb, ww)

                    nonempty = [t for t in range(T)
                                if sum(c_cnt[w2, bb, t] for w2 in range(NW)) > 0]
                    if MODE == "gather_only":
                        nonempty = []
                    # dest-major staging for the batched XBAR transpose:
                    # ys_blk[bt][d, t, f] holds this block's per-type sums.
                    ys_blk = {bt: yspool.tile([128, T, F], H, tag="ys",
                                              name=f"ys{bb}_{bt}")
                              for bt in range(B)}
                    for ti, t in enumerate(nonempty):
                        py = pypool.tile([128, B * F], DT, tag="py",
                                         name=f"py{bb}_{t}")
                        tot = sum(int(c_cnt[w2, bb, t]) for w2 in range(NW))
                        done = 0
                        for ww in range(NW):
                            for k in range(int(c_cnt[ww, bb, t])):
                                col = coff(ww, bb, t) + k
                                if MODE == "pe_only":
                                    oh_ap = iota_t[:]
                                else:
                                    oh_t, c0 = ohs[ww]
                                    oh_ap = oh_t[:, col - c0, :]
                                nc.tensor.matmul(
                                    py[:],
                                    lhsT=oh_ap,
                                    rhs=chunk_rhs(ww, bb, col),
                                    start=(done == 0),
                                    stop=(done == tot - 1),
                                )
                                done += 1
                        for bt in range(B):
                            dst = ys_blk[bt][:, t, :]
                            src = py[:, bt * F:(bt + 1) * F]
                            if t % 2 == 0:
                                nc.scalar.copy(out=dst, in_=src)
                            else:
                                nc.vector.tensor_copy(out=dst, in_=src)
                    # types with no edges anywhere in this block: zero the slot
                    for t in range(T):
                        if t not in nonempty:
                            for bt in range(B):
                                nc.vector.memset(ys_blk[bt][:, t, :], 0.0)
                    # batched XBAR: transpose all T [128d,128f] slices at once
                    for bt in range(B):
                        nc.sync.dma_start(
                            out=ytcat[bt][:, :, bb - sb * SB, :],
                            in_=ys_blk[bt][:],
                            transpose=True)

                if MODE == "gather_only":
                    continue
                # batched fold: out^T[f', d] = sum_t W_t^T @ yT_t over 4 blocks
                biasb = evpool.tile([128, sbw * 128], DT, tag="biasb",
                                    name=f"biasb{sb}")
                nc.sync.dma_start(
                    out=biasb[:],
                    in_=bias_d[:, sb * SB * 128:sb * SB * 128 + sbw * 128])
                for bt in range(B):
                    pout = popool.tile([128, sbw * 128], DT, tag="pout",
                                       name=f"pout{sb}_{bt}")
                    for t in range(T):
                        nc.tensor.matmul(
                            pout[:],
                            lhsT=W_tiles[t][:],
                            rhs=ytcat[bt][:, t, :, :],
                            start=(t == 0),
                            stop=(t == T - 1),
                        )
                    out_t = evpool.tile([128, sbw * 128], DT, tag="out",
                                        name=f"out{sb}_{bt}")
                    nc.vector.tensor_add(out=out_t[:], in0=pout[:],
                                         in1=biasb[:])
                    nc.scalar.dma_start(
                        out=out_d[bt, :, sb * SB * 128:sb * SB * 128 + sbw * 128],
                        in_=out_t[:])

    nc.compile()
    return nc


def kernel(x, W, b, adj_values, end_nodes, start_nodes):
    x = np.asarray(x, dtype=np.float32)
    W = np.asarray(W, dtype=np.float32)
    b = np.asarray(b, dtype=np.float32)
    adj_values = np.asarray(adj_values, dtype=np.float32)
    end_nodes = np.asarray(end_nodes)
    start_nodes = np.asarray(start_nodes)

    struct, per_core = _pack_inputs(x, W, b, adj_values, end_nodes, start_nodes)
    nc = build_kernel(struct["c_cnt"], struct["chunk_off"],
                      struct["total_chunks"])
    results = run_bass_kernel_spmd(nc, per_core,
                                   core_ids=list(range(CORES))).results
    out = np.empty((B, N, F), np.float32)
    for c in range(CORES):
        # device output is feature-major [B, F, NPAD]
        out[:, c * NPC:(c + 1) * NPC, :] = \
            results[c]["out"][:, :, :NPC].transpose(0, 2, 1)
    return out



# revision 33
# speedup vs baseline: 1.6105x; 1.6105x over previous
"""ECC / GNN message-passing kernel for 8 Trainium2 NeuronCores (Bass/Tile).

Reference computation (per edge type t of 7):
    h   = x @ W[t] + b[t]                      # [B, N, F] dense projection
    msg = adj[t] * h[:, end_nodes[t], :]       # gather + edge scale
    out[:, start_nodes[t], :] += msg           # scatter-add

This kernel commutes the linear projection past the gather/scatter:
    out = sum_t (S_t @ x) @ W[t] + deg_t (x) b[t]
where S_t is the weighted adjacency (S_t[s,e] = sum adj over edges s<-e) and
deg_t = S_t @ 1. The deg (x) b bias term is precomputed on the host (O(E)
scalar work); everything O(E*F) runs on device.

Sharding: destination (start-node) range partitioning, 12500 dests/core.
Per core:
  - x is stored batch-interleaved fp16 ([N, 2F], 512B rows) so a single
    dma_gather descriptor fetches both batches' features for an end node.
  - dma_gather uses int16 indices, so x is split into 5 windows of 20000
    rows; edges are grouped (window, dest-block, type) and padded to
    128-edge chunks (pad edges get weight 0, spread gather rows).
  - One-hot scatter matrices for ALL chunks of a (dest-block, window) are
    built in 2 broadcast DVE tensor_tensor ops: (iota == start_local) * adj.
  - Scatter-add runs on the TensorEngine in fp16: per chunk and batch, the
    gathered rows [128e, F] (stationary) multiply the one-hot [128e, 128d]
    (moving), accumulating y_T[f, d] per (dest-block, type) in fp32 PSUM.
  - y_T is copied (fp16, Scalar/Vector engines alternating) into a
    4-block-wide staging tile; one batched fold per (superblock, batch)
    accumulates sum_t W_t^T @ y_T over the 7 types into out^T[f, 4*128d]
    (a single PSUM bank), adds the bias, and stores feature-major output
    [B, F, NPAD] which the host transposes back.

fp16 data path with fp32 PSUM accumulation; relative error vs the fp32
reference is ~4e-4 (gate is 2e-2).
"""
import os
import numpy as np

import concourse.bass as bass
import concourse.mybir as mybir
import concourse.tile as tile
from concourse import bacc
from concourse._compat import get_trn_type
from concourse.bass_utils import run_bass_kernel_spmd

B, N, E, F, T = 2, 100000, 400000, 128, 7
CORES = 8
NPC = N // CORES            # 12500 dests per core
NBLK = (NPC + 127) // 128   # 98 dest blocks per core
NPAD = NBLK * 128
NW = 5                      # gather windows (int16 idx limit is 32768 rows)
WS = N // NW                # 20000
CALLCH = int(os.environ.get("K_CALLCH", "20"))  # chunks per dma_gather call


def _pack_inputs(x, W, b, adj_values, end_nodes, start_nodes):
    """Group edges by (core, window, dest-block, type); build the shared
    chunk structure (max over cores => one SPMD program), per-core gather
    indices, one-hot scalar columns, and the bias term."""
    starts = start_nodes.astype(np.int64).ravel()
    ends = end_nodes.astype(np.int64).ravel()
    adj = adj_values.astype(np.float32).ravel()
    types = np.repeat(np.arange(T, dtype=np.int64), E)

    core = starts // NPC
    sl_all = starts % NPC
    blk = sl_all // 128
    sl_blk = sl_all % 128
    win = ends // WS
    idx16 = (ends % WS).astype(np.int16)

    ng_pc = NW * NBLK * T
    gid_local = (win * NBLK + blk) * T + types
    gid = core * ng_pc + gid_local
    counts = np.bincount(gid, minlength=CORES * ng_pc).reshape(CORES, NW, NBLK, T)
    c_cnt = np.ceil(counts.max(axis=0) / 128).astype(np.int64)  # [NW, NBLK, T]

    chunk_off = np.zeros(ng_pc + 1, np.int64)
    np.cumsum(c_cnt.reshape(-1), out=chunk_off[1:])
    total_chunks = int(chunk_off[-1])
    total_slots = total_chunks * 128

    order = np.argsort(gid, kind="stable")
    sorted_key = gid[order]
    newrun = np.empty(len(sorted_key), bool)
    newrun[0] = True
    newrun[1:] = sorted_key[1:] != sorted_key[:-1]
    run_ids = np.cumsum(newrun) - 1
    run_first = np.flatnonzero(newrun)
    rank = np.arange(len(sorted_key)) - run_first[run_ids]
    slot_sorted = chunk_off[gid_local[order]] * 128 + rank

    # Pad slots get weight 0; spread their gather rows across the window so
    # the padding descriptors don't all hammer the same HBM line.
    pad_rows = (np.arange(total_slots, dtype=np.int64) % WS).astype(np.int16)
    idx_flat = np.broadcast_to(pad_rows, (CORES, total_slots)).copy()
    a_flat = np.zeros((CORES, total_slots), np.float16)
    sl_flat = np.zeros((CORES, total_slots), np.float16)
    core_sorted = core[order]
    idx_flat[core_sorted, slot_sorted] = idx16[order]
    a_flat[core_sorted, slot_sorted] = adj[order].astype(np.float16)
    sl_flat[core_sorted, slot_sorted] = sl_blk[order].astype(np.float16)

    # dma_gather index layout: idx j of a call -> [j % 16, col0 + j // 16],
    # replicated across the 8 groups of 16 partitions.
    idx_wrap = idx_flat.reshape(CORES, total_slots // 16, 16).transpose(0, 2, 1)
    idx_arr = np.tile(idx_wrap, (1, 8, 1)).copy()
    sl_cols = sl_flat.reshape(CORES, total_chunks, 128).transpose(0, 2, 1).copy()
    a_cols = a_flat.reshape(CORES, total_chunks, 128).transpose(0, 2, 1).copy()

    bias_full = np.zeros((N, F), np.float32)
    for t in range(T):
        deg = np.bincount(start_nodes[t].astype(np.int64),
                          weights=adj_values[t].astype(np.float64),
                          minlength=N).astype(np.float32)
        bias_full += deg[:, None] * b[t][None, :]
    # Transposed per-core bias [F, NPAD] (output is produced feature-major).
    biasT_pc = np.zeros((CORES, F, NPAD), np.float32)
    biasT_pc[:, :, :NPC] = bias_full.reshape(CORES, NPC, F).transpose(0, 2, 1)

    iota_rows = np.tile(np.arange(128, dtype=np.float16), (128, 1))

    x_il = np.ascontiguousarray(
        np.concatenate([x[0], x[1]], axis=1).astype(np.float16))  # [N, 2F]

    struct = dict(c_cnt=c_cnt, chunk_off=chunk_off, total_chunks=total_chunks)
    per_core = []
    for c in range(CORES):
        per_core.append({
            "x": x_il,
            "Wt": np.ascontiguousarray(W.astype(np.float16)),
            "idx": idx_arr[c],
            "sl": sl_cols[c],
            "av": a_cols[c],
            "bias": biasT_pc[c],
            "iota": iota_rows,
        })
    return struct, per_core


def build_kernel(c_cnt, chunk_off, total_chunks):
    DT = mybir.dt.float32
    H = mybir.dt.float16
    total_slots = total_chunks * 128
    nc = bacc.Bacc(get_trn_type() or "TRN2", target_bir_lowering=False,
                   num_swdge_queues=4)

    x_d = nc.dram_tensor("x", [N, B * F], H, kind="ExternalInput")
    W_d = nc.dram_tensor("Wt", [T, F, F], H, kind="ExternalInput")
    idx_d = nc.dram_tensor("idx", [128, total_slots // 16], mybir.dt.int16,
                           kind="ExternalInput")
    sl_d = nc.dram_tensor("sl", [128, total_chunks], H, kind="ExternalInput")
    a_d = nc.dram_tensor("av", [128, total_chunks], H, kind="ExternalInput")
    bias_d = nc.dram_tensor("bias", [F, NPAD], DT, kind="ExternalInput")
    iota_d = nc.dram_tensor("iota", [128, 128], H, kind="ExternalInput")
    # Output is feature-major [B, F, NPAD]; the host transposes back.
    out_d = nc.dram_tensor("out", [B, F, NPAD], DT, kind="ExternalOutput")

    def coff(ww, bb, tt):
        return int(chunk_off[(ww * NBLK + bb) * T + tt])

    # Per-window gather-call ranges aligned to dest-block boundaries so each
    # block reads exactly one call per window (bounds the live g-tile set).
    calls, call_of_blk = [], []
    for ww in range(NW):
        cs, cmap = [], []
        cur0 = coff(ww, 0, 0)
        for bbx in range(NBLK):
            b1 = coff(ww, bbx + 1, 0) if bbx + 1 < NBLK else (
                coff(ww + 1, 0, 0) if ww + 1 < NW else total_chunks)
            b0 = coff(ww, bbx, 0)
            if b1 - cur0 > CALLCH and b0 > cur0:
                cs.append((cur0, b0))
                cur0 = b0
            cmap.append(len(cs))
        last1 = coff(ww + 1, 0, 0) if ww + 1 < NW else total_chunks
        if last1 > cur0:
            cs.append((cur0, last1))
        cmap = [min(ci, len(cs) - 1) for ci in cmap]
        calls.append(cs)
        call_of_blk.append(cmap)

    G_BUFS = int(os.environ.get("K_GBUFS", "10"))
    MODE = os.environ.get("K_MODE", "full")  # full | gather_only | compute_only
    SB = 4                       # dest blocks folded per batched fold matmul
    NSB = (NBLK + SB - 1) // SB  # superblocks per core
    qrr = [0]

    with tile.TileContext(nc) as tc:
        with (
            tc.tile_pool(name="const", bufs=1) as cpool,
            tc.tile_pool(name="ip", bufs=12) as idxpool,
            tc.tile_pool(name="gp", bufs=G_BUFS) as gpool,
            tc.tile_pool(name="oh", bufs=int(os.environ.get("K_OHBUFS", "10"))) as ohpool,
            tc.tile_pool(name="yt", bufs=4) as ytpool,
            tc.tile_pool(name="ev", bufs=6) as evpool,
            tc.tile_pool(name="py", bufs=int(os.environ.get("K_PYBUFS", "4")), space="PSUM") as pypool,
            tc.tile_pool(name="po", bufs=int(os.environ.get("K_POBUFS", "4")), space="PSUM") as popool,
        ):
            sl_t = cpool.tile([128, total_chunks], H)
            a_t = cpool.tile([128, total_chunks], H)
            iota_t = cpool.tile([128, 128], H)
            nc.sync.dma_start(out=sl_t[:], in_=sl_d[:])
            nc.sync.dma_start(out=a_t[:], in_=a_d[:])
            nc.sync.dma_start(out=iota_t[:], in_=iota_d[:])
            W_tiles = []
            for t in range(T):
                wt = cpool.tile([F, F], H, tag=f"W{t}")
                nc.sync.dma_start(out=wt[:], in_=W_d[t])
                W_tiles.append(wt)

            gt = {}

            fake_g = None
            if MODE == "pe_only":
                fake_g = cpool.tile([128, B * F], H, tag="fakeg")
                nc.sync.dma_start(out=fake_g[:], in_=x_d[0:128, :])

            def emit_call(ww, ci):
                c0, c1 = calls[ww][ci]
                nch = c1 - c0
                L = nch * 128
                if MODE == "pe_only":
                    gt[(ww, ci)] = True
                    return
                g = gpool.tile([128, nch, B * F], H, tag="g",
                               name=f"g{ww}_{ci}")
                if MODE == "compute_only":
                    r0 = (c0 * 128) % (N - L)
                    for ch in range(nch):
                        nc.sync.dma_start(
                            out=g[:, ch, :],
                            in_=x_d[r0 + ch * 128:r0 + (ch + 1) * 128, :])
                else:
                    idxp = idxpool.tile([128, nch * 8], mybir.dt.int16,
                                        tag="idxc", name=f"idxc{ww}_{ci}")
                    nc.sync.dma_start(out=idxp[:], in_=idx_d[:, c0 * 8:c1 * 8])
                    nc.gpsimd.dma_gather(
                        g[:],
                        x_d[ww * WS:(ww + 1) * WS, :],
                        idxp[:],
                        L, L, B * F,
                        queue_num=qrr[0] % 4,
                        single_packet=False,
                    )
                qrr[0] += 1
                gt[(ww, ci)] = g

            def chunk_lhsT(ww, bb_, cglob, bt):
                if MODE == "pe_only":
                    return fake_g[:, bt * F:(bt + 1) * F]
                ci = call_of_blk[ww][bb_]
                c0 = calls[ww][ci][0]
                return gt[(ww, ci)][:, cglob - c0, bt * F:(bt + 1) * F]

            # One-hot tiles are built per (block, window): the chunk columns
            # of one block within one window are contiguous in chunk space,
            # so 2 broadcast tensor_tensor ops build all of them at once.
            def build_oh(bb, ww):
                c0 = coff(ww, bb, 0)
                c1 = coff(ww, bb + 1, 0) if bb + 1 < NBLK else (
                    coff(ww + 1, 0, 0) if ww + 1 < NW else total_chunks)
                nch = c1 - c0
                oh = ohpool.tile([128, nch, 128], H, tag="oh",
                                 name=f"oh{bb}_{ww}")
                shp = (128, nch, 128)
                nc.vector.tensor_tensor(
                    out=oh[:],
                    in0=iota_t[:].unsqueeze(1).broadcast_to(shp),
                    in1=sl_t[:, c0:c1].unsqueeze(2).broadcast_to(shp),
                    op=mybir.AluOpType.is_equal)
                nc.vector.tensor_tensor(
                    out=oh[:], in0=oh[:],
                    in1=a_t[:, c0:c1].unsqueeze(2).broadcast_to(shp),
                    op=mybir.AluOpType.mult)
                return oh, c0

            for sb in range(NSB):
                blks = range(sb * SB, min((sb + 1) * SB, NBLK))
                sbw = len(blks)
                # yTcat[f, t, j, d]: per-type transposed partial sums for the
                # sbw blocks of this superblock, consumed by the batched fold.
                ytcat = {bt: ytpool.tile([F, T, sbw, 128], H, tag="ytc",
                                         name=f"ytc{sb}_{bt}")
                         for bt in range(B)}
                for bb in blks:
                    for ww in range(NW):
                        ci = call_of_blk[ww][bb]
                        if (ww, ci) not in gt:
                            emit_call(ww, ci)
                    ohs = {}
                    if MODE not in ("gather_only", "pe_only"):
                        for ww in range(NW):
                            ohs[ww] = build_oh(bb, ww)

                    nonempty = [t for t in range(T)
                                if sum(c_cnt[w2, bb, t] for w2 in range(NW)) > 0]
                    if MODE == "gather_only":
                        nonempty = []
                    for ti, t in enumerate(nonempty):
                        pys = {bt: pypool.tile([F, 128], DT, tag="py",
                                               name=f"py{bb}_{t}_{bt}")
                               for bt in range(B)}
                        tot = sum(int(c_cnt[w2, bb, t]) for w2 in range(NW))
                        done = 0
                        for ww in range(NW):
                            for k in range(int(c_cnt[ww, bb, t])):
                                col = coff(ww, bb, t) + k
                                if MODE == "pe_only":
                                    oh_ap = iota_t[:]
                                else:
                                    oh_t, c0 = ohs[ww]
                                    oh_ap = oh_t[:, col - c0, :]
                                for bt in range(B):
                                    nc.tensor.matmul(
                                        pys[bt][:],
                                        lhsT=chunk_lhsT(ww, bb, col, bt),
                                        rhs=oh_ap,
                                        start=(done == 0),
                                        stop=(done == tot - 1),
                                    )
                                done += 1
                        for bt in range(B):
                            dst = ytcat[bt][:, t, bb - sb * SB, :]
                            if t % 2 == 0:
                                nc.scalar.copy(out=dst, in_=pys[bt][:])
                            else:
                                nc.vector.tensor_copy(out=dst, in_=pys[bt][:])
                    # types with no edges anywhere in this block: zero the slot
                    for t in range(T):
                        if t not in nonempty:
                            for bt in range(B):
                                nc.vector.memset(
                                    ytcat[bt][:, t, bb - sb * SB, :], 0.0)

                if MODE == "gather_only":
                    continue
                # batched fold: out^T[f', d] = sum_t W_t^T @ yT_t over 4 blocks
                biasb = evpool.tile([128, sbw * 128], DT, tag="biasb",
                                    name=f"biasb{sb}")
                nc.sync.dma_start(
                    out=biasb[:],
                    in_=bias_d[:, sb * SB * 128:sb * SB * 128 + sbw * 128])
                for bt in range(B):
                    pout = popool.tile([128, sbw * 128], DT, tag="pout",
                                       name=f"pout{sb}_{bt}")
                    for t in range(T):
                        nc.tensor.matmul(
                            pout[:],
                            lhsT=W_tiles[t][:],
                            rhs=ytcat[bt][:, t, :, :],
                            start=(t == 0),
                            stop=(t == T - 1),
                        )
                    out_t = evpool.tile([128, sbw * 128], DT, tag="out",
                                        name=f"out{sb}_{bt}")
                    nc.vector.tensor_add(out=out_t[:], in0=pout[:],
                                         in1=biasb[:])
                    nc.scalar.dma_start(
                        out=out_d[bt, :, sb * SB * 128:sb * SB * 128 + sbw * 128],
                        in_=out_t[:])

    nc.compile()
    return nc


def kernel(x, W, b, adj_values, end_nodes, start_nodes):
    x = np.asarray(x, dtype=np.float32)
    W = np.asarray(W, dtype=np.float32)
    b = np.asarray(b, dtype=np.float32)
    adj_values = np.asarray(adj_values, dtype=np.float32)
    end_nodes = np.asarray(end_nodes)
    start_nodes = np.asarray(start_nodes)

    struct, per_core = _pack_inputs(x, W, b, adj_values, end_nodes, start_nodes)
    nc = build_kernel(struct["c_cnt"], struct["chunk_off"],
                      struct["total_chunks"])
    results = run_bass_kernel_spmd(nc, per_core,
                                   core_ids=list(range(CORES))).results
    out = np.empty((B, N, F), np.float32)
    for c in range(CORES):
        # device output is feature-major [B, F, NPAD]
        out[:, c * NPC:(c + 1) * NPC, :] = \
            results[c]["out"][:, :, :NPC].transpose(0, 2, 1)
    return out

